# revision 4
# baseline (speedup 1.0000x reference)
"""ComplEx rhs-scoring kernel for Trainium2 (8 NeuronCores).

scores = Re(<lhs * rel, conj(all_ents)>) = q @ ent_emb.T
where q = [q_re, q_im] (complex product of gathered lhs/rel embeddings).

Strategy (tensor-parallel over candidates):
  - host: gather + complex product -> q [B, K] (tiny, exact fp32),
    transpose to qT [K, B]; transpose ent_emb -> eT [K, N]; split eT
    into 8 column slabs [K, N/8] (one per core); replicate qT.
  - device (per core): scores_slab[b, n] = sum_k qT[k, b] * eT[k, n]
    via PE matmuls: lhsT = qT k-tile [128, 128], rhs = eT chunk
    [128, CW], accumulate K/128 = 8 matmuls into PSUM fp32.
  - host: concat slabs along axis 1 -> [B, N] fp32.
"""

import os
import numpy as np

import concourse.bacc as bacc
import concourse.mybir as mybir
import concourse.tile as tile
from concourse.bass_utils import run_bass_kernel_spmd

N_CORES = 8
B = 1024          # batch (queries)
K = 1024          # contraction dim (2 * rank)
N_ENT = 100000    # candidates
NS = N_ENT // N_CORES  # per-core slab width (12500)
P = 128           # partitions
KT = K // P       # k tiles (8)
BT = B // P       # b tiles (8)
CW = 500          # rhs chunk width (one PSUM bank; 25 even chunks per slab)

_DT = {
    "bf16": mybir.dt.bfloat16,
    "f32r": mybir.dt.float32r,
    "f32": mybir.dt.float32,
}


def build_kernel(dt_name, ns=NS, cw=CW, b=B):
    dt_in = _DT[dt_name]
    f32 = mybir.dt.float32
    nc = bacc.Bacc("TRN2", target_bir_lowering=False, debug=False)

    qT = nc.dram_tensor("qT", [K, b], dt_in, kind="ExternalInput")
    eT = nc.dram_tensor("eT", [K, ns], dt_in, kind="ExternalInput")
    out = nc.dram_tensor("out", [b, ns], f32, kind="ExternalOutput")

    bt = b // P
    # chunk widths: full cw chunks plus one remainder chunk
    widths = [cw] * (ns // cw)
    if ns % cw:
        widths.append(ns % cw)
    offs = [sum(widths[:i]) for i in range(len(widths))]
    n_chunks = len(widths)

    # 3D-AP views: put the 128-partition dim first, keep k/b tile index
    # as a middle dim so a whole chunk moves in ONE dma_start (the sync
    # engine's ~0.8us per-issue cost is the scarce resource here).
    eT_r = eT.rearrange("(kt p) n -> p kt n", p=P)    # [128, KT, ns]
    qT_r = qT.rearrange("(kt p) b -> p kt b", p=P)    # [128, KT, b]
    out_r = out.rearrange("(bt p) n -> p bt n", p=P)  # [128, bt, ns]

    with tile.TileContext(nc) as tc:
        with (
            tc.tile_pool(name="qpool", bufs=1) as qpool,
            tc.tile_pool(name="epool", bufs=4) as epool,
            tc.tile_pool(name="pspool", bufs=8, space="PSUM") as pspool,
            tc.tile_pool(name="opool", bufs=2) as opool,
        ):
            # chunk-0 entities first so the first matmuls aren't gated on
            # the full q load. Chunk 0 is split per-k so the transfers fan
            # out across DMA queues (latency matters here; later chunks
            # are single issues since only throughput matters there).
            # q resident in SBUF, loaded in b-quarters: the first quarter
            # unblocks b-tiles 0..1 while the rest streams in. Issue order
            # matches first consumption: et0[k0], q-quarter0, remaining
            # et0 k-slices, remaining q.
            et0 = epool.tile([P, KT * cw], dt_in, tag="et")
            qsb = qpool.tile([P, KT * b], dt_in)
            qsb_r = qsb.rearrange("p (kt b) -> p kt b", kt=KT)
            bq = b // 4

            kh = KT // 2

            def q_quarter(j):
                # two kt-half DMAs per quarter: single-issue DMAs only
                # reach ~160-300GB/s, a pair fans out across queues
                for h in range(2):
                    nc.sync.dma_start(
                        qsb_r[:, h * kh:(h + 1) * kh, j * bq:(j + 1) * bq],
                        qT_r[:, h * kh:(h + 1) * kh, j * bq:(j + 1) * bq],
                    )

            # warm the PE (HAM clock-gate needs ~3.4us of activity) with
            # dummy matmuls on a memset tile while the first DMAs land
            warm = qpool.tile([P, cw], mybir.dt.bfloat16, name="warm")
            nc.gpsimd.memset(warm[:], 0.0)
            ps_w = pspool.tile([P, cw], f32, tag="ps", name="ps_warm")
            for _ in range(6):
                nc.tensor.matmul(ps_w[:], warm[:, 0:P], warm[:],
                                 start=True, stop=True)

            nc.sync.dma_start(et0[:, 0:cw], eT[0:P, 0:cw])
            q_quarter(0)
            for k in range(1, KT):
                nc.sync.dma_start(
                    et0[:, k * cw:(k + 1) * cw],
                    eT[k * P:(k + 1) * P, 0:cw],
                )
            for j in range(1, 4):
                q_quarter(j)

            for c in range(n_chunks):
                w = widths[c]
                off = offs[c]
                if c == 0:
                    et = et0
                else:
                    et = epool.tile([P, KT * w], dt_in, tag="et", name=f"et{c}")
                    et_v = et.rearrange("p (kt w) -> p kt w", kt=KT)
                    for j in range(2):
                        nc.sync.dma_start(
                            et_v[:, j * kh:(j + 1) * kh, :],
                            eT_r[:, j * kh:(j + 1) * kh, off:off + w],
                        )
                ot = opool.tile([P, bt * w], f32, tag="ot", name=f"ot{c}")
                for bi in range(bt):
                    ps = pspool.tile([P, w], f32, tag="ps", name="ps")
                    for k in range(KT):
                        nc.tensor.matmul(
                            ps[:],
                            qsb[:, k * b + bi * P:k * b + (bi + 1) * P],
                            et[:, k * w:(k + 1) * w],
                            start=(k == 0),
                            stop=(k == KT - 1),
                        )
                    if bi % 2 == 0:
                        nc.vector.tensor_copy(ot[:, bi * w:(bi + 1) * w], ps[:])
                    else:
                        nc.scalar.copy(ot[:, bi * w:(bi + 1) * w], ps[:])
                    ot_h = ot.rearrange("p (bt w) -> p bt w", bt=bt)
                    # writebacks alternate between the gpsimd and scalar DGE
                    # queues (~150GB/s sustained writes saturate one queue and
                    # a backlog builds; two queues keep the drain current) and
                    # never ride the sync queue, whose issues prefetch entity
                    # chunks
                    if c == n_chunks - 1:
                        # last chunk: flush per-b-tile so the final transfer
                        # after the last matmul is 256KB, not a whole pair
                        eng = nc.gpsimd if bi % 2 == 1 else nc.scalar
                        eng.dma_start(
                            out_r[:, bi:bi + 1, off:off + w],
                            ot_h[:, bi:bi + 1, :],
                        )
                    elif bi % 2 == 1:
                        # flush each b-pair as soon as its copies land
                        h0 = bi - 1
                        eng = nc.gpsimd if (bi // 2) % 2 == 0 else nc.scalar
                        eng.dma_start(
                            out_r[:, h0:bi + 1, off:off + w],
                            ot_h[:, h0:bi + 1, :],
                        )
    nc.compile()
    return nc


def _prep_inputs(x, ent_emb, rel_emb, dt_name):
    x = np.asarray(x)
    ent_emb = np.asarray(ent_emb, dtype=np.float32)
    rel_emb = np.asarray(rel_emb, dtype=np.float32)
    r = ent_emb.shape[1] // 2
    lhs = ent_emb[x[:, 0]]
    rel = rel_emb[x[:, 1]]
    lre, lim = lhs[:, :r], lhs[:, r:]
    rre, rim = rel[:, :r], rel[:, r:]
    q = np.empty((x.shape[0], 2 * r), np.float32)
    q[:, :r] = lre * rre - lim * rim
    q[:, r:] = lre * rim + lim * rre

    if dt_name == "bf16":
        import ml_dtypes
        np_dt = ml_dtypes.bfloat16
    else:
        np_dt = np.float32

    qT = np.ascontiguousarray(q.T).astype(np_dt)           # [K, B]
    eT = np.ascontiguousarray(ent_emb.T).astype(np_dt)     # [K, N]
    in_maps = [
        {"qT": qT, "eT": np.ascontiguousarray(eT[:, i * NS:(i + 1) * NS])}
        for i in range(N_CORES)
    ]
    return in_maps


def run(x, ent_emb, rel_emb, dt_name=None, trace=False, **spmd_kwargs):
    dt_name = dt_name or os.environ.get("KERNEL_DT", "bf16")
    nc = build_kernel(dt_name)
    in_maps = _prep_inputs(x, ent_emb, rel_emb, dt_name)
    res = run_bass_kernel_spmd(
        nc, in_maps, list(range(N_CORES)), trace=trace, **spmd_kwargs
    )
    outs = [res.results[i]["out"] for i in range(N_CORES)]
    return np.concatenate(outs, axis=1), res


def kernel(x, ent_emb, rel_emb):
    out, _ = run(x, ent_emb, rel_emb)
    return out



# revision 6
# speedup vs baseline: 1.3447x; 1.3447x over previous
"""ComplEx rhs-scoring kernel for Trainium2 (8 NeuronCores).

scores = Re(<lhs * rel, conj(all_ents)>) = q @ ent_emb.T
where q = [q_re, q_im] (complex product of gathered lhs/rel embeddings).

Strategy (tensor-parallel over candidates):
  - host: gather + complex product -> q [B, K] (tiny, exact fp32),
    transpose to qT [K, B]; transpose ent_emb -> eT [K, N]; split eT
    into 8 column slabs [K, N/8] (one per core); replicate qT.
  - device (per core): scores_slab[b, n] = sum_k qT[k, b] * eT[k, n]
    via PE matmuls: lhsT = qT k-tile [128, 128], rhs = eT chunk
    [128, CW], accumulate K/128 = 8 matmuls into PSUM fp32.
  - host: concat slabs along axis 1 -> [B, N] fp32.
"""

import os
import numpy as np

import concourse.bacc as bacc
import concourse.mybir as mybir
import concourse.tile as tile
from concourse.bass_utils import run_bass_kernel_spmd

N_CORES = 8
B = 1024          # batch (queries)
K = 1024          # contraction dim (2 * rank)
N_ENT = 100000    # candidates
NS = N_ENT // N_CORES  # per-core slab width (12500)
P = 128           # partitions
KT = K // P       # k tiles (8)
BT = B // P       # b tiles (8)
CW = 500          # rhs chunk width (one PSUM bank; 25 even chunks per slab)

_DT = {
    "bf16": mybir.dt.bfloat16,
    "f32r": mybir.dt.float32r,
    "f32": mybir.dt.float32,
}


def build_kernel(dt_name, ns=NS, cw=CW, b=B):
    dt_in = _DT[dt_name]
    f32 = mybir.dt.float32
    nc = bacc.Bacc("TRN2", target_bir_lowering=False, debug=False)

    qT = nc.dram_tensor("qT", [K, b], dt_in, kind="ExternalInput")
    eT = nc.dram_tensor("eT", [K, ns], dt_in, kind="ExternalInput")
    out = nc.dram_tensor("out", [b, ns], f32, kind="ExternalOutput")

    bt = b // P
    # chunk widths: full cw chunks plus one remainder chunk
    widths = [cw] * (ns // cw)
    if ns % cw:
        widths.append(ns % cw)
    offs = [sum(widths[:i]) for i in range(len(widths))]
    n_chunks = len(widths)

    # 3D-AP views: put the 128-partition dim first, keep k/b tile index
    # as a middle dim so a whole chunk moves in ONE dma_start (the sync
    # engine's ~0.8us per-issue cost is the scarce resource here).
    eT_r = eT.rearrange("(kt p) n -> p kt n", p=P)    # [128, KT, ns]
    qT_r = qT.rearrange("(kt p) b -> p kt b", p=P)    # [128, KT, b]
    out_r = out.rearrange("(bt p) n -> p bt n", p=P)  # [128, bt, ns]

    with tile.TileContext(nc) as tc:
        with (
            tc.tile_pool(name="qpool", bufs=1) as qpool,
            tc.tile_pool(name="epool", bufs=4) as epool,
            tc.tile_pool(name="pspool", bufs=8, space="PSUM") as pspool,
            tc.tile_pool(name="opool", bufs=2) as opool,
        ):
            # chunk-0 entities first so the first matmuls aren't gated on
            # the full q load. Chunk 0 is split per-k so the transfers fan
            # out across DMA queues (latency matters here; later chunks
            # are single issues since only throughput matters there).
            # q resident in SBUF, loaded in b-quarters: the first quarter
            # unblocks b-tiles 0..1 while the rest streams in. Issue order
            # matches first consumption: et0[k0], q-quarter0, remaining
            # et0 k-slices, remaining q.
            et0 = epool.tile([P, KT * cw], dt_in, tag="et")
            qsb = qpool.tile([P, KT * b], dt_in)
            qsb_r = qsb.rearrange("p (kt b) -> p kt b", kt=KT)
            bq = b // 4

            kh = KT // 2

            def q_quarter(j):
                # two kt-half DMAs per quarter: single-issue DMAs only
                # reach ~160-300GB/s, a pair fans out across queues
                for h in range(2):
                    nc.sync.dma_start(
                        qsb_r[:, h * kh:(h + 1) * kh, j * bq:(j + 1) * bq],
                        qT_r[:, h * kh:(h + 1) * kh, j * bq:(j + 1) * bq],
                    )

            # warm the PE (HAM clock-gate needs ~3.4us of activity) with
            # dummy matmuls on a memset tile while the first DMAs land
            warm = qpool.tile([P, cw], mybir.dt.bfloat16, name="warm")
            nc.gpsimd.memset(warm[:], 0.0)
            ps_w = pspool.tile([P, cw], f32, tag="ps", name="ps_warm")
            for _ in range(6):
                nc.tensor.matmul(ps_w[:], warm[:, 0:P], warm[:],
                                 start=True, stop=True)

            nc.sync.dma_start(et0[:, 0:cw], eT[0:P, 0:cw])
            q_quarter(0)
            for k in range(1, KT):
                nc.sync.dma_start(
                    et0[:, k * cw:(k + 1) * cw],
                    eT[k * P:(k + 1) * P, 0:cw],
                )
            for j in range(1, 4):
                q_quarter(j)

            for c in range(n_chunks):
                w = widths[c]
                off = offs[c]
                if c == 0:
                    et = et0
                else:
                    et = epool.tile([P, KT * w], dt_in, tag="et", name=f"et{c}")
                    et_v = et.rearrange("p (kt w) -> p kt w", kt=KT)
                    for j in range(2):
                        nc.sync.dma_start(
                            et_v[:, j * kh:(j + 1) * kh, :],
                            eT_r[:, j * kh:(j + 1) * kh, off:off + w],
                        )
                ot = opool.tile([P, bt * w], f32, tag="ot", name=f"ot{c}")
                for bi in range(bt):
                    ps = pspool.tile([P, w], f32, tag="ps", name="ps")
                    for k in range(KT):
                        nc.tensor.matmul(
                            ps[:],
                            qsb[:, k * b + bi * P:k * b + (bi + 1) * P],
                            et[:, k * w:(k + 1) * w],
                            start=(k == 0),
                            stop=(k == KT - 1),
                        )
                    if bi % 2 == 0:
                        nc.vector.tensor_copy(ot[:, bi * w:(bi + 1) * w], ps[:])
                    else:
                        nc.scalar.copy(ot[:, bi * w:(bi + 1) * w], ps[:])
                    ot_h = ot.rearrange("p (bt w) -> p bt w", bt=bt)
                    # writebacks alternate between the gpsimd and scalar DGE
                    # queues (~150GB/s sustained writes saturate one queue and
                    # a backlog builds; two queues keep the drain current) and
                    # never ride the sync queue, whose issues prefetch entity
                    # chunks
                    if c == n_chunks - 1:
                        # last chunk: flush per-b-tile so the final transfer
                        # after the last matmul is 256KB, not a whole pair
                        eng = nc.gpsimd if bi % 2 == 1 else nc.scalar
                        eng.dma_start(
                            out_r[:, bi:bi + 1, off:off + w],
                            ot_h[:, bi:bi + 1, :],
                        )
                    elif bi % 2 == 1:
                        # flush each b-pair as soon as its copies land
                        h0 = bi - 1
                        eng = nc.gpsimd if (bi // 2) % 2 == 0 else nc.scalar
                        eng.dma_start(
                            out_r[:, h0:bi + 1, off:off + w],
                            ot_h[:, h0:bi + 1, :],
                        )
    nc.compile()
    return nc


KF = 256              # k-rows carried in fp8e4m3 via one DoubleRow super-tile
KTB = (K - KF) // P   # bf16 k-tiles (6)


def build_kernel_hyb(ns=NS, cw=CW, b=B):
    """Hybrid-precision kernel: the first KF=256 contraction rows run as a
    single fp8e4m3 DoubleRow matmul (0.5 PE cycles/row, contraction 2x128
    per instruction); the remaining 768 rows run as 6 bf16 matmuls. All
    operands are pre-scaled by powers of two on the host (fp8's 2^-6 min
    normal can't reach the ~1e-6 input magnitudes) and the host descales
    the fp32 output, so psum accumulation needs no on-device fixup."""
    f8 = mybir.dt.float8e4
    bf = mybir.dt.bfloat16
    f32 = mybir.dt.float32
    DR = mybir.MatmulPerfMode.DoubleRow
    nc = bacc.Bacc("TRN2", target_bir_lowering=False, debug=False)

    qT8 = nc.dram_tensor("qT8", [KF, b], f8, kind="ExternalInput")
    qT16 = nc.dram_tensor("qT16", [K - KF, b], bf, kind="ExternalInput")
    eT8 = nc.dram_tensor("eT8", [KF, ns], f8, kind="ExternalInput")
    eT16 = nc.dram_tensor("eT16", [K - KF, ns], bf, kind="ExternalInput")
    out = nc.dram_tensor("out", [b, ns], f32, kind="ExternalOutput")

    bt = b // P
    widths = [cw] * (ns // cw)
    if ns % cw:
        widths.append(ns % cw)
    offs = [sum(widths[:i]) for i in range(len(widths))]
    n_chunks = len(widths)

    # DoubleRow operand layout [p, two, n]: plane i pairs weight plane i
    # with ifmap plane i, i.e. contraction index = i*128 + p
    eT8_r = eT8.rearrange("(two p) n -> p two n", p=P)     # [128, 2, ns]
    qT8_r = qT8.rearrange("(two p) b -> p two b", p=P)     # [128, 2, b]
    eT16_r = eT16.rearrange("(kt p) n -> p kt n", p=P)     # [128, KTB, ns]
    qT16_r = qT16.rearrange("(kt p) b -> p kt b", p=P)     # [128, KTB, b]
    out_r = out.rearrange("(bt p) n -> p bt n", p=P)

    with tile.TileContext(nc) as tc:
        with (
            tc.tile_pool(name="qpool", bufs=1) as qpool,
            tc.tile_pool(name="epool", bufs=4) as epool,
            tc.tile_pool(name="pspool", bufs=8, space="PSUM") as pspool,
            tc.tile_pool(name="opool", bufs=2) as opool,
        ):
            q8sb = qpool.tile([P, 2 * b], f8)
            q8_v = q8sb.rearrange("p (two b) -> p two b", two=2)
            q16sb = qpool.tile([P, KTB * b], bf)
            q16_v = q16sb.rearrange("p (kt b) -> p kt b", kt=KTB)
            et8_0 = epool.tile([P, 2 * cw], f8, tag="et8")
            et8_0v = et8_0.rearrange("p (two w) -> p two w", two=2)
            et16_0 = epool.tile([P, KTB * cw], bf, tag="et16")
            et16_0v = et16_0.rearrange("p (kt w) -> p kt w", kt=KTB)
            bq = b // 4
            kh = KTB // 2

            warm = qpool.tile([P, cw], bf, name="warm")
            nc.gpsimd.memset(warm[:], 0.0)
            ps_w = pspool.tile([P, cw], f32, tag="ps", name="ps_warm")
            for _ in range(10):
                nc.tensor.matmul(ps_w[:], warm[:, 0:P], warm[:],
                                 start=True, stop=True)

            # issue order matches first consumption: chunk-0 fp8 entities,
            # fp8 q, chunk-0 bf16 entities, bf16 q in b-quarters
            nc.sync.dma_start(et8_0v[:, :, :], eT8_r[:, :, 0:cw])
            nc.sync.dma_start(q8_v[:, :, :], qT8_r[:, :, :])
            for h in range(2):
                nc.sync.dma_start(et16_0v[:, h * kh:(h + 1) * kh, :],
                                  eT16_r[:, h * kh:(h + 1) * kh, 0:cw])
            for j in range(4):
                for h in range(2):
                    nc.sync.dma_start(
                        q16_v[:, h * kh:(h + 1) * kh, j * bq:(j + 1) * bq],
                        qT16_r[:, h * kh:(h + 1) * kh, j * bq:(j + 1) * bq],
                    )

            for c in range(n_chunks):
                w = widths[c]
                off = offs[c]
                if c == 0:
                    et8, et8_v = et8_0, et8_0v
                    et16, et16_v = et16_0, et16_0v
                else:
                    et8 = epool.tile([P, 2 * w], f8, tag="et8", name=f"et8_{c}")
                    et8_v = et8.rearrange("p (two w) -> p two w", two=2)
                    et16 = epool.tile([P, KTB * w], bf, tag="et16",
                                      name=f"et16_{c}")
                    et16_v = et16.rearrange("p (kt w) -> p kt w", kt=KTB)
                    nc.sync.dma_start(et8_v[:, :, :], eT8_r[:, :, off:off + w])
                    for h in range(2):
                        nc.sync.dma_start(
                            et16_v[:, h * kh:(h + 1) * kh, :],
                            eT16_r[:, h * kh:(h + 1) * kh, off:off + w],
                        )
                ot = opool.tile([P, bt * w], f32, tag="ot", name=f"ot{c}")
                ot_h = ot.rearrange("p (bt w) -> p bt w", bt=bt)
                for bi in range(bt):
                    ps = pspool.tile([P, w], f32, tag="ps", name="ps")
                    nc.tensor.matmul(
                        ps[:],
                        q8_v[:, :, bi * P:(bi + 1) * P],
                        et8_v[:, :, :],
                        start=True, stop=False,
                        perf_mode=DR,
                    )
                    for k in range(KTB):
                        nc.tensor.matmul(
                            ps[:],
                            q16sb[:, k * b + bi * P:k * b + (bi + 1) * P],
                            et16[:, k * w:(k + 1) * w],
                            start=False,
                            stop=(k == KTB - 1),
                        )
                    if bi % 2 == 0:
                        nc.vector.tensor_copy(ot[:, bi * w:(bi + 1) * w], ps[:])
                    else:
                        nc.scalar.copy(ot[:, bi * w:(bi + 1) * w], ps[:])
                    if bi % 2 == 1:
                        h0 = bi - 1
                        nc.gpsimd.dma_start(
                            out_r[:, h0:bi + 1, off:off + w],
                            ot_h[:, h0:bi + 1, :],
                        )
    nc.compile()
    return nc


def _prep_inputs_hyb(x, ent_emb, rel_emb):
    import ml_dtypes
    x = np.asarray(x)
    ent_emb = np.asarray(ent_emb, dtype=np.float32)
    rel_emb = np.asarray(rel_emb, dtype=np.float32)
    r = ent_emb.shape[1] // 2
    lhs = ent_emb[x[:, 0]]
    rel = rel_emb[x[:, 1]]
    lre, lim = lhs[:, :r], lhs[:, r:]
    rre, rim = rel[:, :r], rel[:, r:]
    q = np.empty((x.shape[0], 2 * r), np.float32)
    q[:, :r] = lre * rre - lim * rim
    q[:, r:] = lre * rim + lim * rre

    qT = np.ascontiguousarray(q.T)           # [K, B] f32
    eT = np.ascontiguousarray(ent_emb.T)     # [K, N] f32
    # power-of-2 scales put the fp8 operands' std near 1.0; bf16 rounding
    # is scale-invariant for powers of 2, so the same scale is applied to
    # the bf16 rows and divided back out of the output on the host
    a = 2.0 ** np.round(np.log2(1.0 / qT[:KF].std()))
    bs = 2.0 ** np.round(np.log2(1.0 / eT[:KF].std()))
    qT8 = np.ascontiguousarray((qT[:KF] * a).astype(ml_dtypes.float8_e4m3fn))
    qT16 = np.ascontiguousarray((qT[KF:] * a).astype(ml_dtypes.bfloat16))
    eT8 = (eT[:KF] * bs).astype(ml_dtypes.float8_e4m3fn)
    eT16 = (eT[KF:] * bs).astype(ml_dtypes.bfloat16)
    in_maps = [
        {
            "qT8": qT8,
            "qT16": qT16,
            "eT8": np.ascontiguousarray(eT8[:, i * NS:(i + 1) * NS]),
            "eT16": np.ascontiguousarray(eT16[:, i * NS:(i + 1) * NS]),
        }
        for i in range(N_CORES)
    ]
    return in_maps, np.float32(1.0 / (a * bs))


def _prep_inputs(x, ent_emb, rel_emb, dt_name):
    x = np.asarray(x)
    ent_emb = np.asarray(ent_emb, dtype=np.float32)
    rel_emb = np.asarray(rel_emb, dtype=np.float32)
    r = ent_emb.shape[1] // 2
    lhs = ent_emb[x[:, 0]]
    rel = rel_emb[x[:, 1]]
    lre, lim = lhs[:, :r], lhs[:, r:]
    rre, rim = rel[:, :r], rel[:, r:]
    q = np.empty((x.shape[0], 2 * r), np.float32)
    q[:, :r] = lre * rre - lim * rim
    q[:, r:] = lre * rim + lim * rre

    if dt_name == "bf16":
        import ml_dtypes
        np_dt = ml_dtypes.bfloat16
    else:
        np_dt = np.float32

    qT = np.ascontiguousarray(q.T).astype(np_dt)           # [K, B]
    eT = np.ascontiguousarray(ent_emb.T).astype(np_dt)     # [K, N]
    in_maps = [
        {"qT": qT, "eT": np.ascontiguousarray(eT[:, i * NS:(i + 1) * NS])}
        for i in range(N_CORES)
    ]
    return in_maps


def run(x, ent_emb, rel_emb, dt_name=None, trace=False, **spmd_kwargs):
    dt_name = dt_name or os.environ.get("KERNEL_DT", "bf16")
    if dt_name == "hyb8":
        nc = build_kernel_hyb()
        in_maps, descale = _prep_inputs_hyb(x, ent_emb, rel_emb)
    else:
        nc = build_kernel(dt_name)
        in_maps = _prep_inputs(x, ent_emb, rel_emb, dt_name)
        descale = None
    res = run_bass_kernel_spmd(
        nc, in_maps, list(range(N_CORES)), trace=trace, **spmd_kwargs
    )
    outs = [res.results[i]["out"] for i in range(N_CORES)]
    full = np.concatenate(outs, axis=1)
    if descale is not None:
        full *= descale
    return full, res


def kernel(x, ent_emb, rel_emb):
    out, _ = run(x, ent_emb, rel_emb)
    return out



# revision 7
# speedup vs baseline: 1.3555x; 1.0080x over previous
"""ComplEx rhs-scoring kernel for Trainium2 (8 NeuronCores).

scores = Re(<lhs * rel, conj(all_ents)>) = q @ ent_emb.T
where q = [q_re, q_im] (complex product of gathered lhs/rel embeddings).

Strategy (tensor-parallel over candidates):
  - host: gather + complex product -> q [B, K] (tiny, exact fp32),
    transpose to qT [K, B]; transpose ent_emb -> eT [K, N]; split eT
    into 8 column slabs [K, N/8] (one per core); replicate qT.
  - device (per core): scores_slab[b, n] = sum_k qT[k, b] * eT[k, n]
    via PE matmuls: lhsT = qT k-tile [128, 128], rhs = eT chunk
    [128, CW], accumulate K/128 = 8 matmuls into PSUM fp32.
  - host: concat slabs along axis 1 -> [B, N] fp32.
"""

import os
import numpy as np

import concourse.bacc as bacc
import concourse.mybir as mybir
import concourse.tile as tile
from concourse.bass_utils import run_bass_kernel_spmd

N_CORES = 8
B = 1024          # batch (queries)
K = 1024          # contraction dim (2 * rank)
N_ENT = 100000    # candidates
NS = N_ENT // N_CORES  # per-core slab width (12500)
P = 128           # partitions
KT = K // P       # k tiles (8)
BT = B // P       # b tiles (8)
CW = 500          # rhs chunk width (one PSUM bank; 25 even chunks per slab)

_DT = {
    "bf16": mybir.dt.bfloat16,
    "f32r": mybir.dt.float32r,
    "f32": mybir.dt.float32,
}


def build_kernel(dt_name, ns=NS, cw=CW, b=B):
    dt_in = _DT[dt_name]
    f32 = mybir.dt.float32
    nc = bacc.Bacc("TRN2", target_bir_lowering=False, debug=False)

    qT = nc.dram_tensor("qT", [K, b], dt_in, kind="ExternalInput")
    eT = nc.dram_tensor("eT", [K, ns], dt_in, kind="ExternalInput")
    out = nc.dram_tensor("out", [b, ns], f32, kind="ExternalOutput")

    bt = b // P
    # chunk widths: full cw chunks plus one remainder chunk
    widths = [cw] * (ns // cw)
    if ns % cw:
        widths.append(ns % cw)
    offs = [sum(widths[:i]) for i in range(len(widths))]
    n_chunks = len(widths)

    # 3D-AP views: put the 128-partition dim first, keep k/b tile index
    # as a middle dim so a whole chunk moves in ONE dma_start (the sync
    # engine's ~0.8us per-issue cost is the scarce resource here).
    eT_r = eT.rearrange("(kt p) n -> p kt n", p=P)    # [128, KT, ns]
    qT_r = qT.rearrange("(kt p) b -> p kt b", p=P)    # [128, KT, b]
    out_r = out.rearrange("(bt p) n -> p bt n", p=P)  # [128, bt, ns]

    with tile.TileContext(nc) as tc:
        with (
            tc.tile_pool(name="qpool", bufs=1) as qpool,
            tc.tile_pool(name="epool", bufs=4) as epool,
            tc.tile_pool(name="pspool", bufs=8, space="PSUM") as pspool,
            tc.tile_pool(name="opool", bufs=2) as opool,
        ):
            # chunk-0 entities first so the first matmuls aren't gated on
            # the full q load. Chunk 0 is split per-k so the transfers fan
            # out across DMA queues (latency matters here; later chunks
            # are single issues since only throughput matters there).
            # q resident in SBUF, loaded in b-quarters: the first quarter
            # unblocks b-tiles 0..1 while the rest streams in. Issue order
            # matches first consumption: et0[k0], q-quarter0, remaining
            # et0 k-slices, remaining q.
            et0 = epool.tile([P, KT * cw], dt_in, tag="et")
            qsb = qpool.tile([P, KT * b], dt_in)
            qsb_r = qsb.rearrange("p (kt b) -> p kt b", kt=KT)
            bq = b // 4

            kh = KT // 2

            def q_quarter(j):
                # two kt-half DMAs per quarter: single-issue DMAs only
                # reach ~160-300GB/s, a pair fans out across queues
                for h in range(2):
                    nc.sync.dma_start(
                        qsb_r[:, h * kh:(h + 1) * kh, j * bq:(j + 1) * bq],
                        qT_r[:, h * kh:(h + 1) * kh, j * bq:(j + 1) * bq],
                    )

            # warm the PE (HAM clock-gate needs ~3.4us of activity) with
            # dummy matmuls on a memset tile while the first DMAs land
            warm = qpool.tile([P, cw], mybir.dt.bfloat16, name="warm")
            nc.gpsimd.memset(warm[:], 0.0)
            ps_w = pspool.tile([P, cw], f32, tag="ps", name="ps_warm")
            for _ in range(6):
                nc.tensor.matmul(ps_w[:], warm[:, 0:P], warm[:],
                                 start=True, stop=True)

            nc.sync.dma_start(et0[:, 0:cw], eT[0:P, 0:cw])
            q_quarter(0)
            for k in range(1, KT):
                nc.sync.dma_start(
                    et0[:, k * cw:(k + 1) * cw],
                    eT[k * P:(k + 1) * P, 0:cw],
                )
            for j in range(1, 4):
                q_quarter(j)

            for c in range(n_chunks):
                w = widths[c]
                off = offs[c]
                if c == 0:
                    et = et0
                else:
                    et = epool.tile([P, KT * w], dt_in, tag="et", name=f"et{c}")
                    et_v = et.rearrange("p (kt w) -> p kt w", kt=KT)
                    for j in range(2):
                        nc.sync.dma_start(
                            et_v[:, j * kh:(j + 1) * kh, :],
                            eT_r[:, j * kh:(j + 1) * kh, off:off + w],
                        )
                ot = opool.tile([P, bt * w], f32, tag="ot", name=f"ot{c}")
                for bi in range(bt):
                    ps = pspool.tile([P, w], f32, tag="ps", name="ps")
                    for k in range(KT):
                        nc.tensor.matmul(
                            ps[:],
                            qsb[:, k * b + bi * P:k * b + (bi + 1) * P],
                            et[:, k * w:(k + 1) * w],
                            start=(k == 0),
                            stop=(k == KT - 1),
                        )
                    if bi % 2 == 0:
                        nc.vector.tensor_copy(ot[:, bi * w:(bi + 1) * w], ps[:])
                    else:
                        nc.scalar.copy(ot[:, bi * w:(bi + 1) * w], ps[:])
                    ot_h = ot.rearrange("p (bt w) -> p bt w", bt=bt)
                    # writebacks alternate between the gpsimd and scalar DGE
                    # queues (~150GB/s sustained writes saturate one queue and
                    # a backlog builds; two queues keep the drain current) and
                    # never ride the sync queue, whose issues prefetch entity
                    # chunks
                    if c == n_chunks - 1:
                        # last chunk: flush per-b-tile so the final transfer
                        # after the last matmul is 256KB, not a whole pair
                        eng = nc.gpsimd if bi % 2 == 1 else nc.scalar
                        eng.dma_start(
                            out_r[:, bi:bi + 1, off:off + w],
                            ot_h[:, bi:bi + 1, :],
                        )
                    elif bi % 2 == 1:
                        # flush each b-pair as soon as its copies land
                        h0 = bi - 1
                        eng = nc.gpsimd if (bi // 2) % 2 == 0 else nc.scalar
                        eng.dma_start(
                            out_r[:, h0:bi + 1, off:off + w],
                            ot_h[:, h0:bi + 1, :],
                        )
    nc.compile()
    return nc


KF = 256              # k-rows carried in fp8e4m3 via one DoubleRow super-tile
KTB = (K - KF) // P   # bf16 k-tiles (6)


def build_kernel_hyb(ns=NS, cw=CW, b=B):
    """Hybrid-precision kernel: the first KF=256 contraction rows run as a
    single fp8e4m3 DoubleRow matmul (0.5 PE cycles/row, contraction 2x128
    per instruction); the remaining 768 rows run as 6 bf16 matmuls. All
    operands are pre-scaled by powers of two on the host (fp8's 2^-6 min
    normal can't reach the ~1e-6 input magnitudes) and the host descales
    the fp32 output, so psum accumulation needs no on-device fixup."""
    f8 = mybir.dt.float8e4
    bf = mybir.dt.bfloat16
    f32 = mybir.dt.float32
    DR = mybir.MatmulPerfMode.DoubleRow
    nc = bacc.Bacc("TRN2", target_bir_lowering=False, debug=False)

    qT8 = nc.dram_tensor("qT8", [KF, b], f8, kind="ExternalInput")
    qT16 = nc.dram_tensor("qT16", [K - KF, b], bf, kind="ExternalInput")
    eT8 = nc.dram_tensor("eT8", [KF, ns], f8, kind="ExternalInput")
    eT16 = nc.dram_tensor("eT16", [K - KF, ns], bf, kind="ExternalInput")
    out = nc.dram_tensor("out", [b, ns], f32, kind="ExternalOutput")

    bt = b // P
    widths = [cw] * (ns // cw)
    if ns % cw:
        widths.append(ns % cw)
    offs = [sum(widths[:i]) for i in range(len(widths))]
    n_chunks = len(widths)

    # DoubleRow operand layout [p, two, n]: plane i pairs weight plane i
    # with ifmap plane i, i.e. contraction index = i*128 + p
    eT8_r = eT8.rearrange("(two p) n -> p two n", p=P)     # [128, 2, ns]
    qT8_r = qT8.rearrange("(two p) b -> p two b", p=P)     # [128, 2, b]
    eT16_r = eT16.rearrange("(kt p) n -> p kt n", p=P)     # [128, KTB, ns]
    qT16_r = qT16.rearrange("(kt p) b -> p kt b", p=P)     # [128, KTB, b]
    out_r = out.rearrange("(bt p) n -> p bt n", p=P)

    with tile.TileContext(nc) as tc:
        with (
            tc.tile_pool(name="qpool", bufs=1) as qpool,
            tc.tile_pool(name="epool", bufs=4) as epool,
            tc.tile_pool(name="pspool", bufs=8, space="PSUM") as pspool,
            tc.tile_pool(name="opool", bufs=2) as opool,
        ):
            q8sb = qpool.tile([P, 2 * b], f8)
            q8_v = q8sb.rearrange("p (two b) -> p two b", two=2)
            q16sb = qpool.tile([P, KTB * b], bf)
            q16_v = q16sb.rearrange("p (kt b) -> p kt b", kt=KTB)
            et8_0 = epool.tile([P, 2 * cw], f8, tag="et8")
            et8_0v = et8_0.rearrange("p (two w) -> p two w", two=2)
            et16_0 = epool.tile([P, KTB * cw], bf, tag="et16")
            et16_0v = et16_0.rearrange("p (kt w) -> p kt w", kt=KTB)
            bq = b // 4
            kh = KTB // 2

            warm = qpool.tile([P, cw], bf, name="warm")
            nc.gpsimd.memset(warm[:], 0.0)
            ps_w = pspool.tile([P, cw], f32, tag="ps", name="ps_warm")
            for _ in range(10):
                nc.tensor.matmul(ps_w[:], warm[:, 0:P], warm[:],
                                 start=True, stop=True)

            # issue order matches first consumption: chunk-0 fp8 entities,
            # fp8 q, chunk-0 bf16 entities, bf16 q in b-quarters
            nc.sync.dma_start(et8_0v[:, :, :], eT8_r[:, :, 0:cw])
            nc.sync.dma_start(q8_v[:, :, :], qT8_r[:, :, :])
            for h in range(2):
                nc.sync.dma_start(et16_0v[:, h * kh:(h + 1) * kh, :],
                                  eT16_r[:, h * kh:(h + 1) * kh, 0:cw])
            for j in range(4):
                for h in range(2):
                    nc.sync.dma_start(
                        q16_v[:, h * kh:(h + 1) * kh, j * bq:(j + 1) * bq],
                        qT16_r[:, h * kh:(h + 1) * kh, j * bq:(j + 1) * bq],
                    )

            for c in range(n_chunks):
                w = widths[c]
                off = offs[c]
                if c == 0:
                    et8, et8_v = et8_0, et8_0v
                    et16, et16_v = et16_0, et16_0v
                else:
                    et8 = epool.tile([P, 2 * w], f8, tag="et8", name=f"et8_{c}")
                    et8_v = et8.rearrange("p (two w) -> p two w", two=2)
                    et16 = epool.tile([P, KTB * w], bf, tag="et16",
                                      name=f"et16_{c}")
                    et16_v = et16.rearrange("p (kt w) -> p kt w", kt=KTB)
                    nc.sync.dma_start(et8_v[:, :, :], eT8_r[:, :, off:off + w])
                    for h in range(2):
                        nc.sync.dma_start(
                            et16_v[:, h * kh:(h + 1) * kh, :],
                            eT16_r[:, h * kh:(h + 1) * kh, off:off + w],
                        )
                ot = opool.tile([P, bt * w], f32, tag="ot", name=f"ot{c}")
                ot_h = ot.rearrange("p (bt w) -> p bt w", bt=bt)
                # batch the 8 DoubleRow matmuls at the chunk top: the PE
                # pays the normal<->DoubleRow transition once per chunk
                # instead of once per b-tile
                pss = []
                for bi in range(bt):
                    ps = pspool.tile([P, w], f32, tag="ps", name=f"ps{bi}")
                    pss.append(ps)
                    nc.tensor.matmul(
                        ps[:],
                        q8_v[:, :, bi * P:(bi + 1) * P],
                        et8_v[:, :, :],
                        start=True, stop=False,
                        perf_mode=DR,
                    )
                for bi in range(bt):
                    ps = pss[bi]
                    for k in range(KTB):
                        nc.tensor.matmul(
                            ps[:],
                            q16sb[:, k * b + bi * P:k * b + (bi + 1) * P],
                            et16[:, k * w:(k + 1) * w],
                            start=False,
                            stop=(k == KTB - 1),
                        )
                    if bi % 2 == 0:
                        nc.vector.tensor_copy(ot[:, bi * w:(bi + 1) * w], ps[:])
                    else:
                        nc.scalar.copy(ot[:, bi * w:(bi + 1) * w], ps[:])
                    if bi % 2 == 1:
                        h0 = bi - 1
                        nc.gpsimd.dma_start(
                            out_r[:, h0:bi + 1, off:off + w],
                            ot_h[:, h0:bi + 1, :],
                        )
    nc.compile()
    return nc


def _prep_inputs_hyb(x, ent_emb, rel_emb):
    import ml_dtypes
    x = np.asarray(x)
    ent_emb = np.asarray(ent_emb, dtype=np.float32)
    rel_emb = np.asarray(rel_emb, dtype=np.float32)
    r = ent_emb.shape[1] // 2
    lhs = ent_emb[x[:, 0]]
    rel = rel_emb[x[:, 1]]
    lre, lim = lhs[:, :r], lhs[:, r:]
    rre, rim = rel[:, :r], rel[:, r:]
    q = np.empty((x.shape[0], 2 * r), np.float32)
    q[:, :r] = lre * rre - lim * rim
    q[:, r:] = lre * rim + lim * rre

    qT = np.ascontiguousarray(q.T)           # [K, B] f32
    eT = np.ascontiguousarray(ent_emb.T)     # [K, N] f32
    # power-of-2 scales put the fp8 operands' std near 1.0; bf16 rounding
    # is scale-invariant for powers of 2, so the same scale is applied to
    # the bf16 rows and divided back out of the output on the host
    a = 2.0 ** np.round(np.log2(1.0 / qT[:KF].std()))
    bs = 2.0 ** np.round(np.log2(1.0 / eT[:KF].std()))
    qT8 = np.ascontiguousarray((qT[:KF] * a).astype(ml_dtypes.float8_e4m3fn))
    qT16 = np.ascontiguousarray((qT[KF:] * a).astype(ml_dtypes.bfloat16))
    eT8 = (eT[:KF] * bs).astype(ml_dtypes.float8_e4m3fn)
    eT16 = (eT[KF:] * bs).astype(ml_dtypes.bfloat16)
    in_maps = [
        {
            "qT8": qT8,
            "qT16": qT16,
            "eT8": np.ascontiguousarray(eT8[:, i * NS:(i + 1) * NS]),
            "eT16": np.ascontiguousarray(eT16[:, i * NS:(i + 1) * NS]),
        }
        for i in range(N_CORES)
    ]
    return in_maps, np.float32(1.0 / (a * bs))


def _prep_inputs(x, ent_emb, rel_emb, dt_name):
    x = np.asarray(x)
    ent_emb = np.asarray(ent_emb, dtype=np.float32)
    rel_emb = np.asarray(rel_emb, dtype=np.float32)
    r = ent_emb.shape[1] // 2
    lhs = ent_emb[x[:, 0]]
    rel = rel_emb[x[:, 1]]
    lre, lim = lhs[:, :r], lhs[:, r:]
    rre, rim = rel[:, :r], rel[:, r:]
    q = np.empty((x.shape[0], 2 * r), np.float32)
    q[:, :r] = lre * rre - lim * rim
    q[:, r:] = lre * rim + lim * rre

    if dt_name == "bf16":
        import ml_dtypes
        np_dt = ml_dtypes.bfloat16
    else:
        np_dt = np.float32

    qT = np.ascontiguousarray(q.T).astype(np_dt)           # [K, B]
    eT = np.ascontiguousarray(ent_emb.T).astype(np_dt)     # [K, N]
    in_maps = [
        {"qT": qT, "eT": np.ascontiguousarray(eT[:, i * NS:(i + 1) * NS])}
        for i in range(N_CORES)
    ]
    return in_maps


def run(x, ent_emb, rel_emb, dt_name=None, trace=False, **spmd_kwargs):
    dt_name = dt_name or os.environ.get("KERNEL_DT", "bf16")
    if dt_name == "hyb8":
        nc = build_kernel_hyb()
        in_maps, descale = _prep_inputs_hyb(x, ent_emb, rel_emb)
    else:
        nc = build_kernel(dt_name)
        in_maps = _prep_inputs(x, ent_emb, rel_emb, dt_name)
        descale = None
    res = run_bass_kernel_spmd(
        nc, in_maps, list(range(N_CORES)), trace=trace, **spmd_kwargs
    )
    outs = [res.results[i]["out"] for i in range(N_CORES)]
    full = np.concatenate(outs, axis=1)
    if descale is not None:
        full *= descale
    return full, res


def kernel(x, ent_emb, rel_emb):
    out, _ = run(x, ent_emb, rel_emb)
    return out

